# revision 1
# baseline (speedup 1.0000x reference)
"""Trainium2 Bass kernel for the MemoryModule problem.

Computation (per batch b, per l):
    q = Wq @ x_local^T + bq                      (C, D)
    m = Wm @ x_hist^T + bm                       (C, T, D)
    c = Wc @ x_hist^T + bc                       (C, T, D)
    mq[c,t] = sum_d m[c,t,d] q[c,d]
    att = softmax(relu(mq), axis=t)
    o[c,d] = sum_t att[c,t] c[c,t,d]
    out = q + o                                  (C, D)

Key algebraic restructure (so the TensorEngine does all heavy work without
any on-chip transposes of x_hist):

    mq[c,t] = sum_f Wm[c,f] * G[t,f,c] + bm[c]*S[c]
    G[t,f,c] = sum_d x_hist[t,d,f] * q[c,d]   (+ bias terms)
    Substituting q = Wq x_local + bq:
    G = sum_g Wq[c,g] * K[t,f,g] + bq[c] * Z[t,f]
    K[t,f,g] = sum_d x_hist[t,d,f] * x_local[d,g]   <- contract d on PE
    o[c,d]  = sum_{t,f} (att[c,t]*Wc[c,f]) * x_hist[t,d,f] + bc[c]
                                                     <- contract (t,f) on PE

x_hist is fed twice in two host-prepared layouts:
  XT  [d-chunks,128d, l, (t,f)+ones]  - for the d-contraction (scores)
  X2  [l, (t,f), d]                   - for the (t,f)-contraction (apply)
All weight combinations (W2, W2S, QW4b, selectors) are precomputed on host.
Sharding: data-parallel over batch B=8, one batch element per core.
"""

import numpy as np

B, L, T, D, F, C = 8, 12, 36, 1024, 3, 32
TF = T * F  # 108
NCH = D // 128  # 8 d-chunks
NCORES = 8

_CACHE = {}


def _build_program(x2_bf16):
    import concourse.bacc as bacc
    import concourse.mybir as mybir
    import concourse.tile as tile

    f32 = mybir.dt.float32
    x2dt = mybir.dt.bfloat16 if x2_bf16 else f32

    nc = bacc.Bacc("TRN2", target_bir_lowering=False, debug=False,
                   num_devices=NCORES)

    # DRAM layouts exactly match their SBUF tiles (partition-major) so every
    # DMA is long contiguous runs per partition.
    xt4_d = [nc.dram_tensor(f"xt4_{k}", [128, L, TF + 1], f32,
                            kind="ExternalInput") for k in range(NCH)]
    xlp4_d = [nc.dram_tensor(f"xlp4_{k}", [128, L, 4], f32,
                             kind="ExternalInput") for k in range(NCH)]
    x2_d = nc.dram_tensor("x2", [L, TF, D], x2dt, kind="ExternalInput")
    xl4_d = nc.dram_tensor("xl4", [4, L, D], f32, kind="ExternalInput")
    qw4b_d = nc.dram_tensor("qw4b", [4, C], f32, kind="ExternalInput")
    w2_d = nc.dram_tensor("w2", [12, C], f32, kind="ExternalInput")
    w2s_d = nc.dram_tensor("w2s", [4, C], f32, kind="ExternalInput")
    wc_d = nc.dram_tensor("wc", [C, F], f32, kind="ExternalInput")
    sel_d = nc.dram_tensor("sel", [4, F, 12], f32, kind="ExternalInput")
    id_d = nc.dram_tensor("ident", [C, C], f32, kind="ExternalInput")
    out_d = nc.dram_tensor("out", [C, L, D], f32, kind="ExternalOutput")

    AF = mybir.ActivationFunctionType
    AX = mybir.AxisListType
    OP = mybir.AluOpType

    with tile.TileContext(nc) as tc:
        with (
            tc.tile_pool(name="konst", bufs=1) as konst,
            tc.tile_pool(name="x2p", bufs=3) as x2p,
            tc.tile_pool(name="sm", bufs=4) as sm,
            tc.tile_pool(name="outs", bufs=3) as outs,
            tc.tile_pool(name="pss", bufs=3, space="PSUM") as pss,
            tc.tile_pool(name="pso", bufs=2, space="PSUM") as pso,
        ):
            xt4 = []
            xlp4 = []
            for k in range(NCH):
                t_ = konst.tile([128, L, TF + 1], f32, tag=f"xt4_{k}")
                nc.sync.dma_start(out=t_, in_=xt4_d[k][:])
                xt4.append(t_)
                t_ = konst.tile([128, L, 4], f32, tag=f"xlp4_{k}")
                nc.sync.dma_start(out=t_, in_=xlp4_d[k][:])
                xlp4.append(t_)
            xl4 = konst.tile([4, L, D], f32, tag="xl4")
            nc.sync.dma_start(out=xl4, in_=xl4_d[:])
            qw4b = konst.tile([4, C], f32, tag="qw4b")
            nc.sync.dma_start(out=qw4b, in_=qw4b_d[:])
            w2 = konst.tile([12, C], f32, tag="w2")
            nc.sync.dma_start(out=w2, in_=w2_d[:])
            w2s = konst.tile([4, C], f32, tag="w2s")
            nc.sync.dma_start(out=w2s, in_=w2s_d[:])
            wc = konst.tile([C, F], f32, tag="wc")
            nc.sync.dma_start(out=wc, in_=wc_d[:])
            sel = konst.tile([4, F, 12], f32, tag="sel")
            nc.sync.dma_start(out=sel, in_=sel_d[:])
            ident = konst.tile([C, C], f32, tag="ident")
            nc.sync.dma_start(out=ident, in_=id_d[:])

            for l in range(L):
                x2t = x2p.tile([TF, D], x2dt, tag="x2")
                nc.sync.dma_start(out=x2t, in_=x2_d[l])

                # K4'[g',(t,f)+col108] = sum_d xl'[d,g'] * XT[d,(t,f)|1]
                k4p = pss.tile([4, TF + 1], f32, tag="sps")
                for k in range(NCH):
                    nc.tensor.matmul(k4p, lhsT=xlp4[k][:, l, :],
                                     rhs=xt4[k][:, l, :],
                                     start=(k == 0), stop=(k == NCH - 1))
                k4s = sm.tile([4, TF + 1], f32, tag="k4s")
                nc.scalar.copy(out=k4s, in_=k4p)

                # bmS[c] = bm[c]*S[c] via tiny matmul against the ones column
                bmsp = pss.tile([C, 1], f32, tag="sps")
                nc.tensor.matmul(bmsp, lhsT=w2s, rhs=k4s[:, TF:TF + 1],
                                 start=True, stop=True)
                bmss = sm.tile([C, 1], f32, tag="bmss")
                nc.vector.tensor_copy(out=bmss, in_=bmsp)

                # K5[(g'*3+f), t] = K4'[g', t*3+f]  (partition reshuffle via PE)
                k5p = pss.tile([12, T], f32, tag="sps")
                for f in range(F):
                    nc.tensor.matmul(k5p, lhsT=sel[:, f, :],
                                     rhs=k4s[:, f:TF:3],
                                     start=(f == 0), stop=(f == F - 1))
                k5s = sm.tile([12, T], f32, tag="k5s")
                nc.scalar.copy(out=k5s, in_=k5p)

                # mq[c,t] (minus the bm*S term) in one matmul
                mqp = pss.tile([C, T], f32, tag="sps")
                nc.tensor.matmul(mqp, lhsT=w2, rhs=k5s, start=True, stop=True)

                # softmax(relu(mq + bmS))
                relu = sm.tile([C, T], f32, tag="relu")
                nc.scalar.activation(out=relu, in_=mqp, func=AF.Relu,
                                     bias=bmss, scale=1.0)
                nmax = sm.tile([C, 1], f32, tag="nmax")
                nc.vector.tensor_reduce(out=nmax, in_=relu, axis=AX.X,
                                        op=OP.max, negate=True)
                e = sm.tile([C, T], f32, tag="e")
                sume = sm.tile([C, 1], f32, tag="sume")
                nc.scalar.activation(out=e, in_=relu, func=AF.Exp,
                                     bias=nmax, scale=1.0, accum_out=sume)
                rinv = sm.tile([C, 1], f32, tag="rinv")
                nc.vector.reciprocal(out=rinv, in_=sume)
                rw = sm.tile([C, F], f32, tag="rw")
                nc.vector.tensor_scalar_mul(out=rw, in0=wc, scalar1=rinv)

                # attW[c,(t,f)] = att[c,t]*Wc[c,f] (normalized), then
                # transpose on PE to [(t,f), c] for the apply matmul.
                attw = sm.tile([C, TF], f32, tag="attw")
                for f in range(F):
                    nc.scalar.activation(out=attw[:, f:TF:3], in_=e,
                                         func=AF.Copy, scale=rw[:, f:f + 1])
                attp = pss.tile([TF, C], f32, tag="sps")
                nc.tensor.transpose(attp, attw, ident)
                attws = sm.tile([TF, C], x2dt, tag="attws")
                nc.vector.tensor_copy(out=attws, in_=attp)

                # out = q (+bq+bc) then += o, accumulated in PSUM
                outp = pso.tile([C, D], f32, tag="outp")
                for j in range(2):
                    nc.tensor.matmul(outp[:, j * 512:(j + 1) * 512],
                                     lhsT=qw4b,
                                     rhs=xl4[:, l, j * 512:(j + 1) * 512],
                                     start=True, stop=False)
                for j in range(2):
                    nc.tensor.matmul(outp[:, j * 512:(j + 1) * 512],
                                     lhsT=attws,
                                     rhs=x2t[:, j * 512:(j + 1) * 512],
                                     start=False, stop=True)
                outt = outs.tile([C, D], f32, tag="outt")
                if l % 2 == 0:
                    nc.scalar.copy(out=outt, in_=outp)
                else:
                    nc.vector.tensor_copy(out=outt, in_=outp)
                nc.sync.dma_start(out=out_d[:, l, :], in_=outt)

    nc.compile()
    return nc


def _host_prep(x_local, x_hist, Wq, bq, Wm, bm, Wc, bc, x2_bf16):
    """Build per-core input maps (host-side relayout only + tiny weight algebra)."""
    x_local = np.asarray(x_local, np.float32)
    x_hist = np.asarray(x_hist, np.float32)
    Wq = np.asarray(Wq, np.float32)
    bq = np.asarray(bq, np.float32)
    Wm = np.asarray(Wm, np.float32)
    bm = np.asarray(bm, np.float32)
    Wc = np.asarray(Wc, np.float32)
    bc = np.asarray(bc, np.float32)

    # QW4[g,c]: rows Wq^T and bq
    qw4 = np.concatenate([Wq.T, bq[None, :]], 0)          # (4, C)
    qw4b = np.concatenate([Wq.T, (bq + bc)[None, :]], 0)  # (4, C)
    w2 = np.zeros((12, C), np.float32)
    for g in range(4):
        for f in range(3):
            w2[g * 3 + f] = qw4[g] * Wm[:, f]
    w2s = qw4 * bm[None, :]                                # (4, C)
    sel = np.zeros((4, F, 12), np.float32)
    for g in range(4):
        for f in range(F):
            sel[g, f, g * 3 + f] = 1.0
    ident = np.eye(C, dtype=np.float32)

    x2dt = np.dtype("bfloat16") if x2_bf16 else np.float32
    if x2_bf16:
        import ml_dtypes
        x2dt = ml_dtypes.bfloat16

    in_maps = []
    for b in range(B):
        xh = x_hist[b]                       # (L, T, D, F)
        xl = x_local[b]                      # (L, D, F)
        m = {}
        # XT: (D, L, T, F) -> chunks [128, L, TF] + ones col
        xt = np.ascontiguousarray(xh.transpose(2, 0, 1, 3)).reshape(D, L, TF)
        for k in range(NCH):
            blk = np.empty((128, L, TF + 1), np.float32)
            blk[:, :, :TF] = xt[k * 128:(k + 1) * 128]
            blk[:, :, TF] = 1.0
            m[f"xt4_{k}"] = blk
        # xl' : (D, L, F)+ones -> chunks [128, L, 4]
        xlp = np.empty((D, L, 4), np.float32)
        xlp[:, :, :3] = xl.transpose(1, 0, 2)
        xlp[:, :, 3] = 1.0
        for k in range(NCH):
            m[f"xlp4_{k}"] = np.ascontiguousarray(xlp[k * 128:(k + 1) * 128])
        # X2: (L, TF, D)
        m["x2"] = np.ascontiguousarray(xh.transpose(0, 1, 3, 2)).reshape(
            L, TF, D).astype(x2dt)
        # xl4: rows = x_local feature planes + ones, (4, L, D)
        xl4 = np.empty((4, L, D), np.float32)
        xl4[:3] = xl.transpose(2, 0, 1)
        xl4[3] = 1.0
        m["xl4"] = xl4
        m["qw4b"] = qw4b
        m["w2"] = w2
        m["w2s"] = w2s
        m["wc"] = Wc
        m["sel"] = sel
        m["ident"] = ident
        in_maps.append(m)
    return in_maps


X2_BF16 = False


def kernel(x_local, x_hist, Wq, bq, Wm, bm, Wc, bc):
    from concourse.bass_utils import run_bass_kernel_spmd

    key = ("prog", X2_BF16)
    if key not in _CACHE:
        _CACHE[key] = _build_program(X2_BF16)
    nc = _CACHE[key]

    in_maps = _host_prep(x_local, x_hist, Wq, bq, Wm, bm, Wc, bc, X2_BF16)
    res = run_bass_kernel_spmd(nc, in_maps, core_ids=list(range(NCORES)))
    out = np.stack([r["out"] for r in res.results], 0)  # (B, C, L, D)
    return out


# revision 5
# speedup vs baseline: 1.8832x; 1.8832x over previous
"""Trainium2 Bass kernel for the MemoryModule problem.

Computation (per batch b, per l):
    q = Wq @ x_local^T + bq                      (C, D)
    m = Wm @ x_hist^T + bm ; c = Wc @ x_hist^T + bc   (C, T, D)
    mq[c,t] = sum_d m[c,t,d] q[c,d]
    att = softmax(relu(mq), axis=t)
    o[c,d] = sum_t att[c,t] c[c,t,d]
    out = q + o

Algebraic restructure so the TensorEngine does all heavy lifting with no
on-chip transposes of x_hist:
    mq[c,t] = sum_{g,f} Wq[c,g] Wm[c,f] K[t,f,g]
              + bq[c] sum_f Wm[c,f] Z[t,f] + bm[c] S[c]
    K[t,f,g] = sum_d x_hist[t,d,f] x_local[d,g]     <- contract d on PE
    o[c,d]   = sum_{t,f} (att[c,t] Wc[c,f]/denom) x_hist[t,d,f] + bc[c]
                                                    <- contract (t,f) on PE
The d-contraction runs as one full cross-product over all 12 l's
(out [48(l',g'), 12*109(l,t,f)] - PE cost scales only with N, so the
off-diagonal waste is free); per-(l,f) selector matmuls with zeroed rows
both extract the diagonal and reshuffle partitions to K5[(g,f), (l,t)].
All bias terms ride along via ones-columns/rows baked on the host.
x_hist is fed twice in two host-prepared layouts (d-major and (t,f)-major).
Sharding: data-parallel over batch B=8, one batch element per core.
"""

import numpy as np

B, L, T, D, F, C = 8, 12, 36, 1024, 3, 32
TF = T * F          # 108
W = TF + 1          # 109 cols per l-block (ones col for bias sums)
NCH = D // 128      # 8 d-chunks
NCORES = 8

_CACHE = {}


def _build_program(x2_bf16):
    import concourse.bacc as bacc
    import concourse.mybir as mybir
    import concourse.tile as tile

    f32 = mybir.dt.float32
    f32r = mybir.dt.float32r
    x2dt = mybir.dt.bfloat16 if x2_bf16 else f32r

    nc = bacc.Bacc("TRN2", target_bir_lowering=False, debug=False,
                   num_devices=NCORES)

    xt4_d = [nc.dram_tensor(f"xt4_{k}", [128, L, W], f32r,
                            kind="ExternalInput") for k in range(NCH)]
    xlp_d = [nc.dram_tensor(f"xlp_{k}", [128, L * 4], f32r,
                            kind="ExternalInput") for k in range(NCH)]
    x2_d = nc.dram_tensor("x2", [L, TF, D], x2dt, kind="ExternalInput")
    xl4_d = nc.dram_tensor("xl4", [4, L, D], f32r, kind="ExternalInput")
    qw4b_d = nc.dram_tensor("qw4b", [4, C], f32r, kind="ExternalInput")
    w2_d = nc.dram_tensor("w2", [12, C], f32r, kind="ExternalInput")
    selb_d = nc.dram_tensor("selb", [48, L, F, 12], f32, kind="ExternalInput")
    w2sb_d = nc.dram_tensor("w2sb", [48, L, C], f32, kind="ExternalInput")
    wc_d = nc.dram_tensor("wc", [C, F], f32, kind="ExternalInput")
    id_d = nc.dram_tensor("ident", [C, C], f32, kind="ExternalInput")
    out_d = nc.dram_tensor("out", [C, L, D], f32, kind="ExternalOutput")

    AF = mybir.ActivationFunctionType
    AX = mybir.AxisListType
    OP = mybir.AluOpType
    import concourse.bass as bass

    def bcast(ap, extra):
        """Append broadcast/step dims to an AP's free dims."""
        return bass.AP(tensor=ap.tensor, offset=ap.offset, ap=ap.ap + extra)

    with tile.TileContext(nc) as tc:
        with (
            tc.tile_pool(name="konst", bufs=1) as konst,
            tc.tile_pool(name="x2p", bufs=3) as x2p,
            tc.tile_pool(name="sm", bufs=2) as sm,
            tc.tile_pool(name="tl", bufs=4) as tl,
            tc.tile_pool(name="outs", bufs=3) as outs,
        ):
            xt4 = []
            xlp = []
            for k in range(NCH):
                t_ = konst.tile([128, L, W], f32r, tag=f"xt4_{k}")
                nc.sync.dma_start(out=t_, in_=xt4_d[k][:])
                xt4.append(t_)
                t_ = konst.tile([128, L * 4], f32r, tag=f"xlp_{k}")
                nc.sync.dma_start(out=t_, in_=xlp_d[k][:])
                xlp.append(t_)
            xl4 = konst.tile([4, L, D], f32r, tag="xl4")
            nc.sync.dma_start(out=xl4, in_=xl4_d[:])
            qw4b = konst.tile([4, C], f32r, tag="qw4b")
            nc.sync.dma_start(out=qw4b, in_=qw4b_d[:])
            w2 = konst.tile([12, C], f32r, tag="w2")
            nc.sync.dma_start(out=w2, in_=w2_d[:])
            selb = konst.tile([48, L, F, 12], f32, tag="selb")
            nc.sync.dma_start(out=selb, in_=selb_d[:])
            w2sb = konst.tile([48, L, C], f32, tag="w2sb")
            nc.sync.dma_start(out=w2sb, in_=w2sb_d[:])
            wc = konst.tile([C, F], f32, tag="wc")
            nc.sync.dma_start(out=wc, in_=wc_d[:])
            ident = konst.tile([C, C], f32, tag="ident")
            nc.sync.dma_start(out=ident, in_=id_d[:])

            # ---------------- front phase: scores for all l ----------------
            with tc.tile_pool(name="psf", bufs=1, space="PSUM") as psf:
                # K cross-product: [48(l',g'), 12*109(l,(t,f)|1)]
                # padded to 3x512 so each matmul output stays in one PSUM bank
                NS = 3
                NW = L * W // NS  # 436
                k4p = psf.tile([48, NS, 512], f32, tag="k4")
                for j in range(NS):
                    for k in range(NCH):
                        nc.tensor.matmul(
                            k4p[:, j, 0:NW],
                            lhsT=xlp[k][:],
                            rhs=xt4[k][:].rearrange("p l w -> p (l w)")
                                  [:, j * NW:(j + 1) * NW],
                            start=(k == 0), stop=(k == NCH - 1))
                k4s = sm.tile([48, L, W], f32, tag="k4s")
                nc.scalar.copy(
                    out=k4s[:].rearrange("p l w -> p (l w)").rearrange(
                        "p (s n) -> p s n", n=NW),
                    in_=k4p[:, :, 0:NW])

                # K5[(g,f), (l,t)] via selector matmuls (diag extract +
                # partition reshuffle in one op)
                k5p = psf.tile([12, L * T], f32, tag="k5")
                for l in range(L):
                    for f in range(F):
                        nc.tensor.matmul(
                            k5p[:, l * T:(l + 1) * T],
                            lhsT=selb[:, l, f, :],
                            rhs=k4s[:, l, f:TF:3],
                            start=(f == 0), stop=(f == F - 1))
                k5s = sm.tile([12, L * T], f32r, tag="k5s")
                nc.scalar.copy(out=k5s, in_=k5p)

                # bmS[c, l] = bm[c]*S[c, l] from the ones column
                bmsp = psf.tile([C, L], f32, tag="bms")
                for l in range(L):
                    nc.tensor.matmul(bmsp[:, l:l + 1], lhsT=w2sb[:, l, :],
                                     rhs=k4s[:, l, TF:W],
                                     start=True, stop=True)
                bmss = sm.tile([C, L], f32, tag="bmss")
                nc.vector.tensor_copy(out=bmss, in_=bmsp)

                # mq (without bm*S term): [32, (l,t)]
                mqp = psf.tile([C, L * T], f32, tag="mq")
                nc.tensor.matmul(mqp, lhsT=w2[:], rhs=k5s[:],
                                 start=True, stop=True)

                # softmax(relu(mq + bmS)) batched over all l
                mqb = sm.tile([C, L, T], f32, tag="mqb")
                nc.vector.tensor_add(out=mqb, in0=mqp[:].rearrange(
                    "p (l t) -> p l t", t=T), in1=bcast(bmss[:], [[0, T]]))
            relu = sm.tile([C, L, T], f32, tag="relu")
            nc.scalar.activation(out=relu, in_=mqb, func=AF.Relu)
            nmax = sm.tile([C, L], f32, tag="nmax")
            nc.vector.tensor_reduce(out=nmax, in_=relu, axis=AX.X,
                                    op=OP.max, negate=True)
            esub = sm.tile([C, L, T], f32, tag="esub")
            nc.vector.tensor_add(out=esub, in0=relu,
                                 in1=bcast(nmax[:], [[0, T]]))
            eall = sm.tile([C, L, T], f32, tag="eall")
            nc.scalar.activation(out=eall, in_=esub, func=AF.Exp)
            sume = sm.tile([C, L], f32, tag="sume")
            nc.vector.tensor_reduce(out=sume, in_=eall, axis=AX.X, op=OP.add)
            rinv = sm.tile([C, L], f32, tag="rinv")
            nc.vector.reciprocal(out=rinv, in_=sume)
            # rw[c, l, f] = rinv[c,l] * Wc[c,f]
            rw = sm.tile([C, L, F], f32, tag="rw")
            nc.vector.tensor_mul(out=rw, in0=bcast(rinv[:], [[0, F]]),
                                 in1=bass.AP(tensor=wc.tensor, offset=wc.offset,
                                             ap=[wc.ap[0], [0, L], wc.ap[1]]))
            # attw[c, l, t, f] = eall[c,l,t] * rw[c,l,f]
            attw = sm.tile([C, L, T, F], f32, tag="attw")
            nc.vector.tensor_mul(
                out=attw,
                in0=bcast(eall[:], [[0, F]]),
                in1=bass.AP(tensor=rw.tensor, offset=rw.offset,
                            ap=[rw.ap[0], rw.ap[1], [0, T], rw.ap[2]]))

            # ---------------- tail phase: per-l apply ----------------
            with (
                tc.tile_pool(name="pst", bufs=2, space="PSUM") as pst,
                tc.tile_pool(name="pso", bufs=2, space="PSUM") as pso,
            ):
                for l in range(L):
                    x2t = x2p.tile([TF, D], x2dt, tag="x2")
                    nc.sync.dma_start(out=x2t, in_=x2_d[l])

                    attp = pst.tile([TF, C], f32, tag="attp")
                    nc.tensor.transpose(
                        attp, attw[:, l, :, :].rearrange("p t f -> p (t f)"),
                        ident)
                    attws = tl.tile([TF, C], x2dt, tag="attws")
                    nc.vector.tensor_copy(out=attws, in_=attp)

                    outp = pso.tile([C, D], f32, tag="outp")
                    for j in range(2):
                        nc.tensor.matmul(outp[:, j * 512:(j + 1) * 512],
                                         lhsT=qw4b[:],
                                         rhs=xl4[:, l, j * 512:(j + 1) * 512],
                                         start=True, stop=False)
                    for j in range(2):
                        nc.tensor.matmul(outp[:, j * 512:(j + 1) * 512],
                                         lhsT=attws[:],
                                         rhs=x2t[:, j * 512:(j + 1) * 512],
                                         start=False, stop=True)
                    outt = outs.tile([C, D], f32, tag="outt")
                    if l % 2 == 0:
                        nc.scalar.copy(out=outt, in_=outp)
                    else:
                        nc.vector.tensor_copy(out=outt, in_=outp)
                    nc.sync.dma_start(out=out_d[:, l, :], in_=outt)

    nc.compile()
    return nc


def _host_prep(x_local, x_hist, Wq, bq, Wm, bm, Wc, bc, x2_bf16):
    x_local = np.asarray(x_local, np.float32)
    x_hist = np.asarray(x_hist, np.float32)
    Wq = np.asarray(Wq, np.float32)
    bq = np.asarray(bq, np.float32)
    Wm = np.asarray(Wm, np.float32)
    bm = np.asarray(bm, np.float32)
    Wc = np.asarray(Wc, np.float32)
    bc = np.asarray(bc, np.float32)

    qw4 = np.concatenate([Wq.T, bq[None, :]], 0)           # (4, C)
    qw4b = np.concatenate([Wq.T, (bq + bc)[None, :]], 0)   # (4, C)
    w2 = np.zeros((12, C), np.float32)
    for g in range(4):
        for f in range(3):
            w2[g * 3 + f] = qw4[g] * Wm[:, f]
    w2s = qw4 * bm[None, :]                                 # (4, C)
    selb = np.zeros((48, L, F, 12), np.float32)
    w2sb = np.zeros((48, L, C), np.float32)
    for l in range(L):
        for g in range(4):
            for f in range(F):
                selb[4 * l + g, l, f, g * 3 + f] = 1.0
            w2sb[4 * l + g, l, :] = w2s[g]
    ident = np.eye(C, dtype=np.float32)

    if x2_bf16:
        import ml_dtypes
        x2dt = ml_dtypes.bfloat16
    else:
        x2dt = np.float32

    in_maps = []
    for b in range(B):
        xh = x_hist[b]                       # (L, T, D, F)
        xl = x_local[b]                      # (L, D, F)
        m = {}
        xt = np.ascontiguousarray(xh.transpose(2, 0, 1, 3)).reshape(D, L, TF)
        for k in range(NCH):
            blk = np.empty((128, L, W), np.float32)
            blk[:, :, :TF] = xt[k * 128:(k + 1) * 128]
            blk[:, :, TF] = 1.0
            m[f"xt4_{k}"] = blk
        xlp = np.empty((D, L, 4), np.float32)
        xlp[:, :, :3] = xl.transpose(1, 0, 2)
        xlp[:, :, 3] = 1.0
        for k in range(NCH):
            m[f"xlp_{k}"] = np.ascontiguousarray(
                xlp[k * 128:(k + 1) * 128]).reshape(128, L * 4)
        m["x2"] = np.ascontiguousarray(xh.transpose(0, 1, 3, 2)).reshape(
            L, TF, D).astype(x2dt)
        xl4 = np.empty((4, L, D), np.float32)
        xl4[:3] = xl.transpose(2, 0, 1)
        xl4[3] = 1.0
        m["xl4"] = xl4
        m["qw4b"] = qw4b
        m["w2"] = w2
        m["selb"] = selb
        m["w2sb"] = w2sb
        m["wc"] = Wc
        m["ident"] = ident
        in_maps.append(m)
    return in_maps


X2_BF16 = False


def kernel(x_local, x_hist, Wq, bq, Wm, bm, Wc, bc):
    from concourse.bass_utils import run_bass_kernel_spmd

    key = ("prog", X2_BF16)
    if key not in _CACHE:
        _CACHE[key] = _build_program(X2_BF16)
    nc = _CACHE[key]

    in_maps = _host_prep(x_local, x_hist, Wq, bq, Wm, bm, Wc, bc, X2_BF16)
    res = run_bass_kernel_spmd(nc, in_maps, core_ids=list(range(NCORES)))
    out = np.stack([r["out"] for r in res.results], 0)  # (B, C, L, D)
    return out


# revision 7
# speedup vs baseline: 2.3017x; 1.2222x over previous
"""Trainium2 Bass kernel for the MemoryModule problem.

Computation (per batch b, per l):
    q = Wq @ x_local^T + bq                      (C, D)
    m = Wm @ x_hist^T + bm ; c = Wc @ x_hist^T + bc   (C, T, D)
    mq[c,t] = sum_d m[c,t,d] q[c,d]
    att = softmax(relu(mq), axis=t)
    o[c,d] = sum_t att[c,t] c[c,t,d]
    out = q + o

Algebraic restructure so the TensorEngine does all heavy lifting with no
on-chip transposes of x_hist:
    mq[c,t] = sum_{g,f} Wq[c,g] Wm[c,f] K[t,f,g]
              + bq[c] sum_f Wm[c,f] Z[t,f] + bm[c] S[c]
    K[t,f,g] = sum_d x_hist[t,d,f] x_local[d,g]     <- contract d on PE
    o[c,d]   = sum_{t,f} (att[c,t] Wc[c,f]/denom) x_hist[t,d,f] + bc[c]
                                                    <- contract (t,f) on PE
The d-contraction runs as one full cross-product over all 12 l's
(out [48(l',g'), 12*109(l,t,f)] - PE cost scales only with N, so the
off-diagonal waste is free); per-(l,f) selector matmuls with zeroed rows
both extract the diagonal and reshuffle partitions to K5[(g,f), (l,t)].
All bias terms ride along via ones-columns/rows baked on the host.
x_hist is fed twice in two host-prepared layouts (d-major and (t,f)-major).
Sharding: data-parallel over batch B=8, one batch element per core.
"""

import numpy as np

B, L, T, D, F, C = 8, 12, 36, 1024, 3, 32
TF = T * F          # 108
W = TF + 1          # 109 cols per l-block (ones col for bias sums)
NCH = D // 128      # 8 d-chunks
NCORES = 8

_CACHE = {}


def _build_program(x2_bf16):
    import concourse.bacc as bacc
    import concourse.mybir as mybir
    import concourse.tile as tile

    f32 = mybir.dt.float32
    f32r = mybir.dt.float32r
    x2dt = mybir.dt.bfloat16 if x2_bf16 else f32r

    nc = bacc.Bacc("TRN2", target_bir_lowering=False, debug=False,
                   num_devices=NCORES)

    xt4_d = [nc.dram_tensor(f"xt4_{k}", [128, L, W], f32r,
                            kind="ExternalInput") for k in range(NCH)]
    xlp_d = [nc.dram_tensor(f"xlp_{k}", [128, L * 4], f32r,
                            kind="ExternalInput") for k in range(NCH)]
    x2_d = nc.dram_tensor("x2", [L, TF, D], x2dt, kind="ExternalInput")
    xl4_d = nc.dram_tensor("xl4", [4, L, D], f32r, kind="ExternalInput")
    qw4b_d = nc.dram_tensor("qw4b", [4, C], f32r, kind="ExternalInput")
    w2_d = nc.dram_tensor("w2", [12, C], f32r, kind="ExternalInput")
    selb_d = nc.dram_tensor("selb", [48, L, F, 12], f32, kind="ExternalInput")
    w2sb_d = nc.dram_tensor("w2sb", [48, L, C], f32, kind="ExternalInput")
    wc_d = nc.dram_tensor("wc", [C, F], f32, kind="ExternalInput")
    id_d = nc.dram_tensor("ident", [C, C], f32, kind="ExternalInput")
    out_d = nc.dram_tensor("out", [C, L, D], f32, kind="ExternalOutput")

    AF = mybir.ActivationFunctionType
    AX = mybir.AxisListType
    OP = mybir.AluOpType
    import concourse.bass as bass

    def bcast(ap, extra):
        """Append broadcast/step dims to an AP's free dims."""
        return bass.AP(tensor=ap.tensor, offset=ap.offset, ap=ap.ap + extra)

    with tile.TileContext(nc) as tc:
        with (
            tc.tile_pool(name="konst", bufs=1) as konst,
            tc.tile_pool(name="x2p", bufs=1) as x2p,
            tc.tile_pool(name="sm", bufs=2) as sm,
            tc.tile_pool(name="tl", bufs=4) as tl,
            tc.tile_pool(name="outs", bufs=3) as outs,
        ):
            xt4 = []
            xlp = []
            for k in range(NCH):
                t_ = konst.tile([128, L, W], f32r, tag=f"xt4_{k}")
                nc.sync.dma_start(out=t_, in_=xt4_d[k][:])
                xt4.append(t_)
                t_ = konst.tile([128, L * 4], f32r, tag=f"xlp_{k}")
                nc.sync.dma_start(out=t_, in_=xlp_d[k][:])
                xlp.append(t_)
            xl4 = konst.tile([4, L, D], f32r, tag="xl4")
            nc.sync.dma_start(out=xl4, in_=xl4_d[:])
            qw4b = konst.tile([4, C], f32r, tag="qw4b")
            nc.sync.dma_start(out=qw4b, in_=qw4b_d[:])
            w2 = konst.tile([12, C], f32r, tag="w2")
            nc.sync.dma_start(out=w2, in_=w2_d[:])
            selb = konst.tile([48, L, F, 12], f32, tag="selb")
            nc.sync.dma_start(out=selb, in_=selb_d[:])
            w2sb = konst.tile([48, L, C], f32, tag="w2sb")
            nc.sync.dma_start(out=w2sb, in_=w2sb_d[:])
            wc = konst.tile([C, F], f32, tag="wc")
            nc.sync.dma_start(out=wc, in_=wc_d[:])
            ident = konst.tile([C, C], f32, tag="ident")
            nc.sync.dma_start(out=ident, in_=id_d[:])

            # prefetch all per-l apply tiles up front so the DMA engines stay
            # busy during the score phase
            x2ts = []
            for l in range(L):
                x2t = x2p.tile([TF, D], x2dt, tag=f"x2_{l}")
                nc.sync.dma_start(out=x2t, in_=x2_d[l])
                x2ts.append(x2t)

            # ---------------- front phase: scores for all l ----------------
            with tc.tile_pool(name="psf", bufs=1, space="PSUM") as psf:
                # K cross-product: [48(l',g'), 12*109(l,(t,f)|1)]
                # padded to 3x512 so each matmul output stays in one PSUM bank
                NS = 3
                NW = L * W // NS  # 436
                k4p = psf.tile([48, NS, 512], f32, tag="k4")
                for k in range(NCH):
                    for j in range(NS):
                        nc.tensor.matmul(
                            k4p[:, j, 0:NW],
                            lhsT=xlp[k][:],
                            rhs=xt4[k][:].rearrange("p l w -> p (l w)")
                                  [:, j * NW:(j + 1) * NW],
                            start=(k == 0), stop=(k == NCH - 1))
                k4s = sm.tile([48, L, W], f32, tag="k4s")
                nc.scalar.copy(
                    out=k4s[:].rearrange("p l w -> p (l w)").rearrange(
                        "p (s n) -> p s n", n=NW),
                    in_=k4p[:, :, 0:NW])

                # K5[(g,f), (l,t)] via selector matmuls (diag extract +
                # partition reshuffle in one op)
                k5p = psf.tile([12, L * T], f32, tag="k5")
                for l in range(L):
                    for f in range(F):
                        nc.tensor.matmul(
                            k5p[:, l * T:(l + 1) * T],
                            lhsT=selb[:, l, f, :],
                            rhs=k4s[:, l, f:TF:3],
                            start=(f == 0), stop=(f == F - 1))
                k5s = sm.tile([12, L * T], f32r, tag="k5s")
                nc.scalar.copy(out=k5s, in_=k5p)

                # bmS[c, l] = bm[c]*S[c, l] from the ones column
                bmsp = psf.tile([C, L], f32, tag="bms")
                for l in range(L):
                    nc.tensor.matmul(bmsp[:, l:l + 1], lhsT=w2sb[:, l, :],
                                     rhs=k4s[:, l, TF:W],
                                     start=True, stop=True)
                bmss = sm.tile([C, L], f32, tag="bmss")
                nc.vector.tensor_copy(out=bmss, in_=bmsp)

                # mq (without bm*S term): [32, (l,t)]
                mqp = psf.tile([C, L * T], f32, tag="mq")
                nc.tensor.matmul(mqp, lhsT=w2[:], rhs=k5s[:],
                                 start=True, stop=True)

                # softmax(relu(mq + bmS)) batched over all l
                mqb = sm.tile([C, L, T], f32, tag="mqb")
                nc.vector.tensor_add(out=mqb, in0=mqp[:].rearrange(
                    "p (l t) -> p l t", t=T), in1=bcast(bmss[:], [[0, T]]))
            relu = sm.tile([C, L, T], f32, tag="relu")
            nc.scalar.activation(out=relu, in_=mqb, func=AF.Relu)
            nmax = sm.tile([C, L], f32, tag="nmax")
            nc.vector.tensor_reduce(out=nmax, in_=relu, axis=AX.X,
                                    op=OP.max, negate=True)
            esub = sm.tile([C, L, T], f32, tag="esub")
            nc.vector.tensor_add(out=esub, in0=relu,
                                 in1=bcast(nmax[:], [[0, T]]))
            eall = sm.tile([C, L, T], f32, tag="eall")
            nc.scalar.activation(out=eall, in_=esub, func=AF.Exp)
            sume = sm.tile([C, L], f32, tag="sume")
            nc.vector.tensor_reduce(out=sume, in_=eall, axis=AX.X, op=OP.add)
            rinv = sm.tile([C, L], f32, tag="rinv")
            nc.vector.reciprocal(out=rinv, in_=sume)
            # rw[c, l, f] = rinv[c,l] * Wc[c,f]
            rw = sm.tile([C, L, F], f32, tag="rw")
            nc.vector.tensor_mul(out=rw, in0=bcast(rinv[:], [[0, F]]),
                                 in1=bass.AP(tensor=wc.tensor, offset=wc.offset,
                                             ap=[wc.ap[0], [0, L], wc.ap[1]]))
            # attw[c, l, t, f] = eall[c,l,t] * rw[c,l,f]
            attw = sm.tile([C, L, T, F], f32, tag="attw")
            nc.vector.tensor_mul(
                out=attw,
                in0=bcast(eall[:], [[0, F]]),
                in1=bass.AP(tensor=rw.tensor, offset=rw.offset,
                            ap=[rw.ap[0], rw.ap[1], [0, T], rw.ap[2]]))

            # ---------------- tail phase: per-l apply ----------------
            with (
                tc.tile_pool(name="pst", bufs=2, space="PSUM") as pst,
                tc.tile_pool(name="pso", bufs=2, space="PSUM") as pso,
            ):
                for l in range(L):
                    x2t = x2ts[l]

                    attp = pst.tile([TF, C], f32, tag="attp")
                    nc.tensor.transpose(
                        attp, attw[:, l, :, :].rearrange("p t f -> p (t f)"),
                        ident)
                    attws = tl.tile([TF, C], x2dt, tag="attws")
                    nc.vector.tensor_copy(out=attws, in_=attp)

                    outp = pso.tile([C, D], f32, tag="outp")
                    for j in range(2):
                        nc.tensor.matmul(outp[:, j * 512:(j + 1) * 512],
                                         lhsT=qw4b[:],
                                         rhs=xl4[:, l, j * 512:(j + 1) * 512],
                                         start=True, stop=False)
                    for j in range(2):
                        nc.tensor.matmul(outp[:, j * 512:(j + 1) * 512],
                                         lhsT=attws[:],
                                         rhs=x2t[:, j * 512:(j + 1) * 512],
                                         start=False, stop=True)
                    outt = outs.tile([C, D], f32, tag="outt")
                    if l % 2 == 0:
                        nc.scalar.copy(out=outt, in_=outp)
                    else:
                        nc.vector.tensor_copy(out=outt, in_=outp)
                    nc.sync.dma_start(out=out_d[:, l, :], in_=outt)

    nc.compile()
    return nc


def _host_prep(x_local, x_hist, Wq, bq, Wm, bm, Wc, bc, x2_bf16):
    x_local = np.asarray(x_local, np.float32)
    x_hist = np.asarray(x_hist, np.float32)
    Wq = np.asarray(Wq, np.float32)
    bq = np.asarray(bq, np.float32)
    Wm = np.asarray(Wm, np.float32)
    bm = np.asarray(bm, np.float32)
    Wc = np.asarray(Wc, np.float32)
    bc = np.asarray(bc, np.float32)

    qw4 = np.concatenate([Wq.T, bq[None, :]], 0)           # (4, C)
    qw4b = np.concatenate([Wq.T, (bq + bc)[None, :]], 0)   # (4, C)
    w2 = np.zeros((12, C), np.float32)
    for g in range(4):
        for f in range(3):
            w2[g * 3 + f] = qw4[g] * Wm[:, f]
    w2s = qw4 * bm[None, :]                                 # (4, C)
    selb = np.zeros((48, L, F, 12), np.float32)
    w2sb = np.zeros((48, L, C), np.float32)
    for l in range(L):
        for g in range(4):
            for f in range(F):
                selb[4 * l + g, l, f, g * 3 + f] = 1.0
            w2sb[4 * l + g, l, :] = w2s[g]
    ident = np.eye(C, dtype=np.float32)

    if x2_bf16:
        import ml_dtypes
        x2dt = ml_dtypes.bfloat16
    else:
        x2dt = np.float32

    in_maps = []
    for b in range(B):
        xh = x_hist[b]                       # (L, T, D, F)
        xl = x_local[b]                      # (L, D, F)
        m = {}
        xt = np.ascontiguousarray(xh.transpose(2, 0, 1, 3)).reshape(D, L, TF)
        for k in range(NCH):
            blk = np.empty((128, L, W), np.float32)
            blk[:, :, :TF] = xt[k * 128:(k + 1) * 128]
            blk[:, :, TF] = 1.0
            m[f"xt4_{k}"] = blk
        xlp = np.empty((D, L, 4), np.float32)
        xlp[:, :, :3] = xl.transpose(1, 0, 2)
        xlp[:, :, 3] = 1.0
        for k in range(NCH):
            m[f"xlp_{k}"] = np.ascontiguousarray(
                xlp[k * 128:(k + 1) * 128]).reshape(128, L * 4)
        m["x2"] = np.ascontiguousarray(xh.transpose(0, 1, 3, 2)).reshape(
            L, TF, D).astype(x2dt)
        xl4 = np.empty((4, L, D), np.float32)
        xl4[:3] = xl.transpose(2, 0, 1)
        xl4[3] = 1.0
        m["xl4"] = xl4
        m["qw4b"] = qw4b
        m["w2"] = w2
        m["selb"] = selb
        m["w2sb"] = w2sb
        m["wc"] = Wc
        m["ident"] = ident
        in_maps.append(m)
    return in_maps


X2_BF16 = True


def kernel(x_local, x_hist, Wq, bq, Wm, bm, Wc, bc):
    from concourse.bass_utils import run_bass_kernel_spmd

    key = ("prog", X2_BF16)
    if key not in _CACHE:
        _CACHE[key] = _build_program(X2_BF16)
    nc = _CACHE[key]

    in_maps = _host_prep(x_local, x_hist, Wq, bq, Wm, bm, Wc, bc, X2_BF16)
    res = run_bass_kernel_spmd(nc, in_maps, core_ids=list(range(NCORES)))
    out = np.stack([r["out"] for r in res.results], 0)  # (B, C, L, D)
    return out
